# revision 17
# baseline (speedup 1.0000x reference)
"""DGI (Deep Graph Infomax) Trainium2 kernel — v2.

Strategy (8 NeuronCores, one shared SPMD program):
  - Nodes sharded by destination: core c owns dst nodes [c*N/8, (c+1)*N/8).
  - Aggregate-then-multiply: z = PReLU((A_hat @ x) @ W + b).  The x-space
    aggregation needs no precomputed xw, so gathers start immediately.
  - Paired table tab[i] = [x[i] | x[perm[i]]] (f16, 1KB rows).  dma_gather
    cost is row-count-bound (~8ns/row, flat 256B..1KB), so one gather per
    edge fetches BOTH passes' source rows.  One shared weighted one-hot
    (fused is_equal+mult on DVE) and one PE matmul [128e,128d]^T@[128e,512]
    accumulate [agg1|agg2] per dst tile in PSUM.
  - Gathers round-robin over 4 SWDGE queues (~2x DMA overlap).
  - Epilogue per dst tile: PSUM->SBUF f16, 2+2 PE transposes, (aggT @ W)
    k-accumulated, bias+PReLU on DVE; z1/z2 tiles stored f16 in SBUF;
    z1 column-sums accumulate for the summary.
  - summary = sigmoid(mean over all z1) via ones-matmul column reduce +
    1KB AllReduce; wsum = disc_W @ summary on PE; broadcast via K=1 matmul.
  - pos/neg = z . wsum per tile on DVE; per-core [128, DT] outputs,
    host unshards.
"""

import os

import numpy as np

_P = 128
_LO = 32768
_C = 8


def _build_streams(sidx, ed, ew, C, NS, DT):
    """Per-core gather/weight/dstl streams with shared tile structure.

    Returns (idx_sbuf [C,128,n_et*8] i16, w_sbuf [C,128,n_et] f32,
             dl_sbuf [C,128,n_et] f32, Tmax [DT,2], off_tiles [DT,2], n_et)
    """
    core = ed // NS
    ldst = ed - core * NS
    dt = ldst // _P
    dstl = ldst % _P
    cls = (sidx >= _LO).astype(np.int64)

    gid = (core * DT + dt) * 2 + cls
    NG = C * DT * 2
    cnt = np.bincount(gid, minlength=NG).reshape(C, DT, 2)
    T = -(-cnt // _P)
    Tmax = T.max(axis=0)
    flat = Tmax.reshape(-1)
    off_tiles = np.concatenate([[0], np.cumsum(flat)[:-1]]).reshape(DT, 2)
    n_et = int(flat.sum())

    order = np.argsort(gid, kind="stable")
    sorted_gid = gid[order]
    g_starts = np.concatenate(
        [[0], np.cumsum(np.bincount(sorted_gid, minlength=NG))[:-1]]
    )
    rank = np.arange(order.size) - g_starts[sorted_gid]
    g_dt = (sorted_gid // 2) % DT
    g_cls = sorted_gid % 2
    pos = off_tiles[g_dt, g_cls] * _P + rank
    core_s = sorted_gid // (DT * 2)

    L = n_et * _P
    idx16 = np.zeros((C, L), np.int16)
    wv = np.zeros((C, L), np.float16)
    dl = np.full((C, L), -1.0, np.float16)  # int16-able dstl, -1 = pad
    sidx_s = sidx[order]
    idx16[core_s, pos] = (sidx_s - g_cls * _LO).astype(np.int16)
    wv[core_s, pos] = ew[order].astype(np.float16)
    dl[core_s, pos] = dstl[order].astype(np.float16)

    idx_w = idx16.reshape(C, L // 16, 16).transpose(0, 2, 1)
    idx_sbuf = np.ascontiguousarray(np.tile(idx_w, (1, 8, 1)))

    # host-precomputed weighted one-hots: [C, 128e, n_et*128d] f16
    dli = dl.astype(np.int16)
    stw = np.zeros((C, n_et, _P, _P), np.float16)  # [c, tile, e, dstl]
    cc, ll = np.nonzero(dli >= 0)
    tt = ll // _P
    ee = ll % _P
    stw[cc, tt, ee, dli[cc, ll].astype(np.int64)] = wv[cc, ll]
    stw_sbuf = np.ascontiguousarray(
        stw.transpose(0, 2, 1, 3).reshape(C, _P, n_et * _P)
    )
    return idx_sbuf, stw_sbuf, Tmax, off_tiles, n_et


def kernel(x, W, b, a, disc_W, edge_index, perm):
    import bass_rust
    import concourse.bacc as bacc
    import concourse.mybir as mybir
    import concourse.tile as tile
    from concourse.bass_utils import run_bass_kernel_spmd

    x = np.asarray(x)
    W = np.asarray(W)
    b = np.asarray(b, np.float32)
    a = np.asarray(a, np.float32)
    disc_W = np.asarray(disc_W, np.float32)
    ei = np.asarray(edge_index, np.int64)
    perm_np = np.asarray(perm, np.int64)

    N, F = x.shape
    H = W.shape[1]
    C = _C
    NS = N // C
    DT = -(-NS // _P)
    LAST = NS - (DT - 1) * _P
    F2 = 2 * F  # paired row width
    f16 = mybir.dt.float16
    f32 = mybir.dt.float32
    NQ = 4  # SWDGE queues
    USE_ACT = os.environ.get("KV_ACT", "1") == "1"
    USE_TTR = os.environ.get("KV_TTR", "0") == "1"
    GBUFS = int(os.environ.get("KV_GBUFS", "2"))

    # ---- host preprocessing -------------------------------------------
    src = ei[0]
    dst = ei[1]
    deg = (np.bincount(dst, minlength=N) + 1.0).astype(np.float32)
    dinv = (1.0 / np.sqrt(deg)).astype(np.float32)
    ew = dinv[src] * dinv[dst]

    # balanced node->slot remap: deal nodes (sorted by in-edge count, desc)
    # serpentine over C*DT bins of 128 slots -> near-equal edges per tile
    NBINS = C * DT
    NSL = DT * _P  # slots per core
    cnt_in = np.bincount(dst, minlength=N)
    order = np.argsort(-cnt_in, kind="stable")
    ii = np.arange(N, dtype=np.int64)
    rnd = ii // NBINS
    pos_in_rnd = ii % NBINS
    bins = np.where(rnd % 2 == 0, pos_in_rnd, NBINS - 1 - pos_in_rnd)
    slot_sorted = (bins // DT) * NSL + (bins % DT) * _P + rnd
    slot_of_node = np.empty(N, np.int64)
    slot_of_node[order] = slot_sorted
    node_at_slot = np.full(C * NSL, -1, np.int64)
    node_at_slot[slot_of_node] = np.arange(N, dtype=np.int64)

    dst_s = slot_of_node[dst]
    i1, s1, T1, O1, n_et = _build_streams(src, dst_s, ew, C, NSL, DT)
    a_val = float(np.asarray(a).reshape(-1)[0])
    occ = np.zeros((C, _P, DT), np.float16)
    stwself = np.zeros((C, DT, _P, _P), np.float16)
    dsq = dinv * dinv
    nas = node_at_slot.reshape(C, DT, _P)
    for c in range(C):
        for dti in range(DT):
            nn = nas[c, dti]
            valid = nn >= 0
            idxs = np.nonzero(valid)[0]
            stwself[c, dti, idxs, idxs] = dsq[nn[idxs]].astype(np.float16)
            occ[c, valid, dti] = 1.0
    stwself = np.ascontiguousarray(
        stwself.transpose(0, 2, 1, 3).reshape(C, _P, DT * _P)
    )

    x_f16 = x.astype(np.float16)
    tab = np.ascontiguousarray(np.concatenate([x_f16, x_f16[perm_np]], axis=1))
    tab_self = np.zeros((C, DT * _P, 2 * F), np.float16)
    flat = node_at_slot.reshape(C, DT * _P)
    for c in range(C):
        valid = flat[c] >= 0
        tab_self[c, valid] = tab[flat[c][valid]]
    W_f16 = np.ascontiguousarray(W.astype(np.float16))
    dwT = np.ascontiguousarray(disc_W.T.astype(np.float32))
    iota4_np = np.tile(np.arange(_P, dtype=np.float16)[None, :], (_P, 4))
    ident16_np = np.eye(_P, dtype=np.float16)

    max_Tl = max(int(T1[:, 0].max()), 1)
    max_Th = max(int(T1[:, 1].max()), 1)

    # ---- device program -----------------------------------------------
    nc = bacc.Bacc(
        "TRN2", target_bir_lowering=False, debug=False, num_devices=C,
        num_swdge_queues=NQ,
        dynamic_dma_scratch_size=int(os.environ.get("KV_SCR", "32768")),
    )

    t_tab = nc.dram_tensor("tab", [N, F2], f16, kind="ExternalInput")
    t_W = nc.dram_tensor("w16", [F, H], f16, kind="ExternalInput")
    t_b = nc.dram_tensor("bvec", [H], f32, kind="ExternalInput")
    t_dwT = nc.dram_tensor("dwT", [H, H], f32, kind="ExternalInput")

    t_mask = nc.dram_tensor("mask16", [_P, DT], f16, kind="ExternalInput")
    t_ident = nc.dram_tensor("ident_in", [_P, _P], f32, kind="ExternalInput")
    t_ident16 = nc.dram_tensor("ident16", [_P, _P], f16, kind="ExternalInput")
    t_i1 = nc.dram_tensor("idx1", [_P, n_et * 8], mybir.dt.int16, kind="ExternalInput")
    t_stw = nc.dram_tensor("stw1", [_P, n_et * _P], f16, kind="ExternalInput")
    t_tabself = nc.dram_tensor("tabself", [DT * _P, F2], f16, kind="ExternalInput")
    t_stwself = nc.dram_tensor("stwself", [_P, DT * _P], f16, kind="ExternalInput")

    t_pos = nc.dram_tensor("pos_out", [_P, DT], f32, kind="ExternalOutput")
    t_neg = nc.dram_tensor("neg_out", [_P, DT], f32, kind="ExternalOutput")

    t_ar_in = nc.dram_tensor("ar_in", [H], f32)
    t_ar_out = nc.dram_tensor("ar_out", [H], f32, addr_space="Shared")

    tab_lo = t_tab[0:_LO, :]
    tab_hi = t_tab[_LO:N, :]

    with tile.TileContext(nc) as tc:
        import contextlib

        ctx = contextlib.ExitStack()
        consts = ctx.enter_context(tc.tile_pool(name="consts", bufs=1))
        glo = ctx.enter_context(tc.tile_pool(name="glo", bufs=GBUFS))
        ghi = ctx.enter_context(tc.tile_pool(name="ghi", bufs=GBUFS))
        stp = ctx.enter_context(tc.tile_pool(name="stp", bufs=4))
        aggps = ctx.enter_context(tc.tile_pool(name="aggps", bufs=2, space="PSUM"))
        trps = ctx.enter_context(tc.tile_pool(name="trps", bufs=2, space="PSUM"))
        zps = ctx.enter_context(tc.tile_pool(name="zps", bufs=2, space="PSUM"))
        ep = ctx.enter_context(tc.tile_pool(name="ep", bufs=3))
        misc = ctx.enter_context(tc.tile_pool(name="misc", bufs=2))
        miscps = ctx.enter_context(tc.tile_pool(name="miscps", bufs=1, space="PSUM"))
        csps = ctx.enter_context(tc.tile_pool(name="csps", bufs=1, space="PSUM"))

        # ---- constants ----
        W0 = consts.tile([_P, H], f16, tag="W0")
        W1 = consts.tile([_P, H], f16, tag="W1")
        nc.sync.dma_start(W0[:], t_W[0:_P, :])
        nc.sync.dma_start(W1[:], t_W[_P : 2 * _P, :])
        stwself_sb = consts.tile([_P, DT * _P], f16, tag="stwself")
        nc.sync.dma_start(stwself_sb[:], t_stwself[:])
        ident16 = consts.tile([_P, _P], f16, tag="ident16")
        nc.sync.dma_start(ident16[:], t_ident16[:])
        mask16 = consts.tile([_P, DT], f16, tag="mask16")
        nc.sync.dma_start(mask16[:], t_mask[:])
        b_sb = consts.tile([1, H], f32, tag="b_sb")
        nc.sync.dma_start(b_sb[:], t_b[None, :])
        dwT0 = consts.tile([_P, H], f32, tag="dwT0")
        dwT1 = consts.tile([_P, H], f32, tag="dwT1")
        nc.sync.dma_start(dwT0[:], t_dwT[0:_P, :])
        nc.sync.dma_start(dwT1[:], t_dwT[_P : 2 * _P, :])
        ones_row = consts.tile([1, _P], f32, tag="ones_row")
        nc.vector.memset(ones_row[:], 1.0)
        ones_col = consts.tile([_P, 1], f32, tag="ones_col")
        nc.vector.memset(ones_col[:], 1.0)

        # ---- stream loads ----
        i1_sb = consts.tile([_P, n_et * 8], mybir.dt.int16, tag="i1")
        nc.sync.dma_start(i1_sb[:], t_i1[:])

        # ---- persistent z tiles + summary accumulator ----
        zbuf1 = consts.tile([_P, DT * H], f16, tag="zbuf1")
        zbuf2 = consts.tile([_P, DT * H], f16, tag="zbuf2")
        cs_acc = csps.tile([1, H], f32, tag="cs_acc")

        qctr = [0]

        # ---- main sweep: one pass over dst tiles serves both encodings --
        for dti in range(DT):
            Tl, Th = int(T1[dti, 0]), int(T1[dti, 1])
            gl = gh = None
            CH = int(os.environ.get("KV_CH", "4"))  # gather chunk (tiles)
            gs = ep.tile([_P, F2], f16, tag="gs")
            nc.sync.dma_start(gs[:], t_tabself[dti * _P : (dti + 1) * _P, :])
            if Tl:
                o = int(O1[dti, 0])
                gl = glo.tile([_P, max_Tl, F2], f16, tag="gl")
                for c0 in range(0, Tl, CH):
                    c1 = min(Tl, c0 + CH)
                    nc.gpsimd.dma_gather(
                        gl[:, c0:c1, :],
                        tab_lo,
                        i1_sb[:, 8 * (o + c0) : 8 * (o + c1)],
                        (c1 - c0) * _P,
                        (c1 - c0) * _P,
                        F2,
                        single_packet=((c1 - c0) * _P <= 1024),
                        queue_num=qctr[0] % NQ,
                    )
                    qctr[0] += 1
            if Th:
                o = int(O1[dti, 1])
                gh = ghi.tile([_P, max_Th, F2], f16, tag="gh")
                for c0 in range(0, Th, CH):
                    c1 = min(Th, c0 + CH)
                    nc.gpsimd.dma_gather(
                        gh[:, c0:c1, :],
                        tab_hi,
                        i1_sb[:, 8 * (o + c0) : 8 * (o + c1)],
                        (c1 - c0) * _P,
                        (c1 - c0) * _P,
                        F2,
                        single_packet=((c1 - c0) * _P <= 1024),
                        queue_num=qctr[0] % NQ,
                    )
                    qctr[0] += 1

            ps = aggps.tile([_P, F2], f32, tag="aggps")
            n_mm = Tl + Th + 1
            # self-loop contribution: precomputed diag(dinv^2) one-hot
            nc.tensor.matmul(
                ps[:], stwself_sb[:, dti * _P : (dti + 1) * _P], gs[:],
                start=True, stop=False,
            )
            k = 1
            for Tn, g, o0 in ((Tl, gl, int(O1[dti, 0])), (Th, gh, int(O1[dti, 1]))):
                j = 0
                while j < Tn:
                    pw = min(8, Tn - j)
                    t = o0 + j
                    stw = stp.tile([_P, 8 * _P], f16, tag="stw")
                    nc.sync.dma_start(
                        stw[:, : pw * _P], t_stw[:, t * _P : (t + pw) * _P]
                    )
                    for q in range(pw):
                        nc.tensor.matmul(
                            ps[:],
                            stw[:, (q * _P) : (q + 1) * _P],
                            g[:, j + q, :],
                            start=False,
                            stop=(k == n_mm - 1),
                        )
                        k += 1
                    j += pw

            # epilogue: agg [128, 512] = [agg1 | agg2] -> z1, z2 tiles
            agg_sb = ep.tile([_P, F2], f16, tag="agg_sb")
            nc.any.tensor_copy(agg_sb[:], ps[:])
            zp = zps.tile([_P, F2], f32, tag="zp")
            for half in range(2):
                nc.tensor.matmul(
                    zp[:, half * H : (half + 1) * H], ones_row[:], b_sb[:],
                    start=True, stop=False,
                )
            for k4 in range(4):
                tp = trps.tile([_P, _P], f16, tag="trps")
                nc.tensor.transpose(
                    tp[:], agg_sb[:, k4 * _P : (k4 + 1) * _P], ident16[:]
                )
                aggT = ep.tile([_P, _P], f16, tag="aggT")
                nc.any.tensor_copy(aggT[:], tp[:])
                half = 0 if k4 < 2 else 1
                Wk = W0 if (k4 % 2) == 0 else W1
                nc.tensor.matmul(
                    zp[:, half * H : (half + 1) * H], aggT[:], Wk[:],
                    start=False, stop=((k4 % 2) == 1),
                )

            # PReLU straight to f16 stores (scalar engine)
            nc.scalar.activation(
                zbuf1[:, dti * H : (dti + 1) * H], zp[:, 0:H],
                mybir.ActivationFunctionType.Prelu, alpha=a_val,
            )
            nc.scalar.activation(
                zbuf2[:, dti * H : (dti + 1) * H], zp[:, H:F2],
                mybir.ActivationFunctionType.Prelu, alpha=a_val,
            )
            # column-sum accumulate on PE (occupancy mask kills empty slots)
            nc.tensor.matmul(
                cs_acc[:], mask16[:, dti : dti + 1],
                zbuf1[:, dti * H : (dti + 1) * H],
                start=(dti == 0), stop=(dti == DT - 1),
            )

        # ---- summary + AllReduce ----
        cs_sb = misc.tile([1, H], f32, tag="cs_sb")
        nc.vector.tensor_copy(cs_sb[:], cs_acc[:])
        nc.sync.dma_start(t_ar_in[None, :], cs_sb[:])
        nc.gpsimd.collective_compute(
            "AllReduce",
            mybir.AluOpType.add,
            replica_groups=[list(range(C))],
            ins=[t_ar_in[:]],
            outs=[t_ar_out[:]],
        )
        sums_sb = misc.tile([1, H], f32, tag="sums_sb")
        nc.sync.dma_start(sums_sb[:], t_ar_out[None, :])
        summ_sb = misc.tile([1, H], f32, tag="summ_sb")
        nc.scalar.activation(
            summ_sb[:], sums_sb[:], mybir.ActivationFunctionType.Sigmoid,
            scale=1.0 / N,
        )

        # ---- wsum = disc_W @ summary ----
        ident = consts.tile([_P, _P], f32, tag="ident")
        nc.sync.dma_start(ident[:], t_ident[:])
        sT = misc.tile([_P, 2], f32, tag="sT")
        for c_i in range(2):
            tp = miscps.tile([_P, _P], f32, tag="mps")
            nc.tensor.transpose(
                tp[:, 0:1],
                summ_sb[0:1, c_i * _P : (c_i + 1) * _P],
                ident[0:1, 0:1],
            )
            nc.vector.tensor_copy(sT[:, c_i : c_i + 1], tp[:, 0:1])
        ws_ps = miscps.tile([1, H], f32, tag="mps")
        nc.tensor.matmul(ws_ps[:], sT[:, 0:1], dwT0[:], start=True, stop=False)
        nc.tensor.matmul(ws_ps[:], sT[:, 1:2], dwT1[:], start=False, stop=True)
        ws_sb = misc.tile([1, H], f32, tag="ws_sb")
        nc.vector.tensor_copy(ws_sb[:], ws_ps[:])
        wb_ps = miscps.tile([_P, H], f32, tag="mps")
        nc.tensor.matmul(wb_ps[:], ones_row[:], ws_sb[:], start=True, stop=True)
        wsum_bc = consts.tile([_P, H], f16, tag="wsum_bc")
        nc.vector.tensor_copy(wsum_bc[:], wb_ps[:])

        # ---- pos/neg dots ----
        pos_acc = consts.tile([_P, DT], f32, tag="pos_acc")
        neg_acc = consts.tile([_P, DT], f32, tag="neg_acc")
        DB = 7  # dot batch (DT=49 = 7*7)
        scratch = misc.tile([_P, DB, H], f16, tag="scratch")
        for zbuf, acc in ((zbuf1, pos_acc), (zbuf2, neg_acc)):
            for d0 in range(0, DT, DB):
                k = min(DB, DT - d0)
                zv = zbuf[:, d0 * H : (d0 + k) * H].rearrange(
                    "p (t h) -> p t h", t=k
                )
                nc.vector.tensor_tensor(
                    scratch[:, :k, :], zv,
                    wsum_bc[:].rearrange("p (o h) -> p o h", o=1).to_broadcast([_P, k, H]),
                    mybir.AluOpType.mult,
                )
                nc.vector.reduce_sum(
                    acc[:, d0 : d0 + k], scratch[:, :k, :], bass_rust.AxisListType.X
                )

        nc.sync.dma_start(t_pos[:], pos_acc[:])
        nc.sync.dma_start(t_neg[:], neg_acc[:])
        ctx.close()

    nc.compile()

    in_maps = []
    for c in range(C):
        in_maps.append(
            {
                "tab": tab,
                "w16": W_f16,
                "bvec": b,
                "dwT": dwT,
                "mask16": occ[c],
                "ident_in": np.eye(_P, dtype=np.float32),
                "ident16": ident16_np,
                "idx1": i1[c],
                "stw1": s1[c],
                "tabself": tab_self[c],
                "stwself": stwself[c],
            }
        )

    if os.environ.get("KERNEL_SIM", "0") == "1":
        from concourse import bass_interp

        sim = bass_interp.MultiCoreSim(nc, C)
        for c in range(C):
            for k, v in in_maps[c].items():
                sim.cores[c].tensor(k)[:] = v
        sim.simulate()
        results = [
            {
                "pos_out": np.array(sim.cores[c].tensor("pos_out")),
                "neg_out": np.array(sim.cores[c].tensor("neg_out")),
            }
            for c in range(C)
        ]
    else:
        trace = os.environ.get("KERNEL_TRACE", "0") == "1"
        kw = {}
        if trace:
            kw["trace"] = True
        res = run_bass_kernel_spmd(nc, in_maps, core_ids=list(range(C)), **kw)
        kernel.last_result = res
        results = res.results

    pos_s = np.concatenate(
        [results[c]["pos_out"].T.reshape(-1) for c in range(C)]
    )
    neg_s = np.concatenate(
        [results[c]["neg_out"].T.reshape(-1) for c in range(C)]
    )
    return pos_s[slot_of_node].astype(np.float32), neg_s[slot_of_node].astype(
        np.float32
    )


# revision 18
# speedup vs baseline: 1.2098x; 1.2098x over previous
"""DGI (Deep Graph Infomax) Trainium2 kernel — v2.

Strategy (8 NeuronCores, one shared SPMD program):
  - Nodes sharded by destination: core c owns dst nodes [c*N/8, (c+1)*N/8).
  - Aggregate-then-multiply: z = PReLU((A_hat @ x) @ W + b).  The x-space
    aggregation needs no precomputed xw, so gathers start immediately.
  - Paired table tab[i] = [x[i] | x[perm[i]]] (f16, 1KB rows).  dma_gather
    cost is row-count-bound (~8ns/row, flat 256B..1KB), so one gather per
    edge fetches BOTH passes' source rows.  One shared weighted one-hot
    (fused is_equal+mult on DVE) and one PE matmul [128e,128d]^T@[128e,512]
    accumulate [agg1|agg2] per dst tile in PSUM.
  - Gathers round-robin over 4 SWDGE queues (~2x DMA overlap).
  - Epilogue per dst tile: PSUM->SBUF f16, 2+2 PE transposes, (aggT @ W)
    k-accumulated, bias+PReLU on DVE; z1/z2 tiles stored f16 in SBUF;
    z1 column-sums accumulate for the summary.
  - summary = sigmoid(mean over all z1) via ones-matmul column reduce +
    1KB AllReduce; wsum = disc_W @ summary on PE; broadcast via K=1 matmul.
  - pos/neg = z . wsum per tile on DVE; per-core [128, DT] outputs,
    host unshards.
"""

import os

import numpy as np

_P = 128
_LO = 32768
_C = 8


def _build_streams(sidx, ed, ew, C, NS, DT):
    """Per-core gather/weight/dstl streams with shared tile structure.

    Returns (idx_sbuf [C,128,n_et*8] i16, w_sbuf [C,128,n_et] f32,
             dl_sbuf [C,128,n_et] f32, Tmax [DT,2], off_tiles [DT,2], n_et)
    """
    core = ed // NS
    ldst = ed - core * NS
    dt = ldst // _P
    dstl = ldst % _P
    cls = (sidx >= _LO).astype(np.int64)

    gid = (core * DT + dt) * 2 + cls
    NG = C * DT * 2
    cnt = np.bincount(gid, minlength=NG).reshape(C, DT, 2)
    T = -(-cnt // _P)
    Tmax = T.max(axis=0)
    flat = Tmax.reshape(-1)
    off_tiles = np.concatenate([[0], np.cumsum(flat)[:-1]]).reshape(DT, 2)
    n_et = int(flat.sum())

    order = np.argsort(gid, kind="stable")
    sorted_gid = gid[order]
    g_starts = np.concatenate(
        [[0], np.cumsum(np.bincount(sorted_gid, minlength=NG))[:-1]]
    )
    rank = np.arange(order.size) - g_starts[sorted_gid]
    g_dt = (sorted_gid // 2) % DT
    g_cls = sorted_gid % 2
    pos = off_tiles[g_dt, g_cls] * _P + rank
    core_s = sorted_gid // (DT * 2)

    L = n_et * _P
    idx16 = np.zeros((C, L), np.int16)
    wv = np.zeros((C, L), np.float16)
    dl = np.full((C, L), -1.0, np.float16)  # int16-able dstl, -1 = pad
    sidx_s = sidx[order]
    idx16[core_s, pos] = (sidx_s - g_cls * _LO).astype(np.int16)
    wv[core_s, pos] = ew[order].astype(np.float16)
    dl[core_s, pos] = dstl[order].astype(np.float16)

    idx_w = idx16.reshape(C, L // 16, 16).transpose(0, 2, 1)
    idx_sbuf = np.ascontiguousarray(np.tile(idx_w, (1, 8, 1)))
    w_sbuf = np.ascontiguousarray(wv.reshape(C, n_et, _P).transpose(0, 2, 1))
    dl_sbuf = np.ascontiguousarray(dl.reshape(C, n_et, _P).transpose(0, 2, 1))
    return idx_sbuf, w_sbuf, dl_sbuf, Tmax, off_tiles, n_et


def kernel(x, W, b, a, disc_W, edge_index, perm):
    import bass_rust
    import concourse.bacc as bacc
    import concourse.mybir as mybir
    import concourse.tile as tile
    from concourse.bass_utils import run_bass_kernel_spmd

    x = np.asarray(x)
    W = np.asarray(W)
    b = np.asarray(b, np.float32)
    a = np.asarray(a, np.float32)
    disc_W = np.asarray(disc_W, np.float32)
    ei = np.asarray(edge_index, np.int64)
    perm_np = np.asarray(perm, np.int64)

    N, F = x.shape
    H = W.shape[1]
    C = _C
    NS = N // C
    DT = -(-NS // _P)
    LAST = NS - (DT - 1) * _P
    F2 = 2 * F  # paired row width
    f16 = mybir.dt.float16
    f32 = mybir.dt.float32
    NQ = 4  # SWDGE queues
    USE_ACT = os.environ.get("KV_ACT", "1") == "1"
    USE_TTR = os.environ.get("KV_TTR", "0") == "1"
    GBUFS = int(os.environ.get("KV_GBUFS", "2"))

    # ---- host preprocessing -------------------------------------------
    src = ei[0]
    dst = ei[1]
    deg = (np.bincount(dst, minlength=N) + 1.0).astype(np.float32)
    dinv = (1.0 / np.sqrt(deg)).astype(np.float32)
    ew = dinv[src] * dinv[dst]

    # balanced node->slot remap: deal nodes (sorted by in-edge count, desc)
    # serpentine over C*DT bins of 128 slots -> near-equal edges per tile
    NBINS = C * DT
    NSL = DT * _P  # slots per core
    cnt_in = np.bincount(dst, minlength=N)
    order = np.argsort(-cnt_in, kind="stable")
    ii = np.arange(N, dtype=np.int64)
    rnd = ii // NBINS
    pos_in_rnd = ii % NBINS
    bins = np.where(rnd % 2 == 0, pos_in_rnd, NBINS - 1 - pos_in_rnd)
    slot_sorted = (bins // DT) * NSL + (bins % DT) * _P + rnd
    slot_of_node = np.empty(N, np.int64)
    slot_of_node[order] = slot_sorted
    node_at_slot = np.full(C * NSL, -1, np.int64)
    node_at_slot[slot_of_node] = np.arange(N, dtype=np.int64)

    dst_s = slot_of_node[dst]
    i1, w1, d1, T1, O1, n_et = _build_streams(src, dst_s, ew, C, NSL, DT)
    a_val = float(np.asarray(a).reshape(-1)[0])
    occ = np.zeros((C, _P, DT), np.float16)
    stwself = np.zeros((C, DT, _P, _P), np.float16)
    dsq = dinv * dinv
    nas = node_at_slot.reshape(C, DT, _P)
    for c in range(C):
        for dti in range(DT):
            nn = nas[c, dti]
            valid = nn >= 0
            idxs = np.nonzero(valid)[0]
            stwself[c, dti, idxs, idxs] = dsq[nn[idxs]].astype(np.float16)
            occ[c, valid, dti] = 1.0
    stwself = np.ascontiguousarray(
        stwself.transpose(0, 2, 1, 3).reshape(C, _P, DT * _P)
    )

    x_f16 = x.astype(np.float16)
    tab = np.ascontiguousarray(np.concatenate([x_f16, x_f16[perm_np]], axis=1))
    tab_self = np.zeros((C, DT * _P, 2 * F), np.float16)
    flat = node_at_slot.reshape(C, DT * _P)
    for c in range(C):
        valid = flat[c] >= 0
        tab_self[c, valid] = tab[flat[c][valid]]
    W_f16 = np.ascontiguousarray(W.astype(np.float16))
    dwT = np.ascontiguousarray(disc_W.T.astype(np.float32))
    iota4_np = np.tile(np.arange(_P, dtype=np.float16)[None, :], (_P, 4))
    ident16_np = np.eye(_P, dtype=np.float16)
    iota4_np = np.tile(np.arange(_P, dtype=np.float16)[None, :], (_P, 4))

    max_Tl = max(int(T1[:, 0].max()), 1)
    max_Th = max(int(T1[:, 1].max()), 1)

    # ---- device program -----------------------------------------------
    nc = bacc.Bacc(
        "TRN2", target_bir_lowering=False, debug=False, num_devices=C,
        num_swdge_queues=NQ,
        dynamic_dma_scratch_size=int(os.environ.get("KV_SCR", "32768")),
    )

    t_tab = nc.dram_tensor("tab", [N, F2], f16, kind="ExternalInput")
    t_W = nc.dram_tensor("w16", [F, H], f16, kind="ExternalInput")
    t_b = nc.dram_tensor("bvec", [H], f32, kind="ExternalInput")
    t_dwT = nc.dram_tensor("dwT", [H, H], f32, kind="ExternalInput")

    t_mask = nc.dram_tensor("mask16", [_P, DT], f16, kind="ExternalInput")
    t_ident = nc.dram_tensor("ident_in", [_P, _P], f32, kind="ExternalInput")
    t_ident16 = nc.dram_tensor("ident16", [_P, _P], f16, kind="ExternalInput")
    t_i1 = nc.dram_tensor("idx1", [_P, n_et * 8], mybir.dt.int16, kind="ExternalInput")
    t_w1 = nc.dram_tensor("wgt1", [_P, n_et], f16, kind="ExternalInput")
    t_d1 = nc.dram_tensor("dstl1", [_P, n_et], f16, kind="ExternalInput")
    t_iota4 = nc.dram_tensor("iota4", [_P, 4 * _P], f16, kind="ExternalInput")
    t_tabself = nc.dram_tensor("tabself", [DT * _P, F2], f16, kind="ExternalInput")
    t_stwself = nc.dram_tensor("stwself", [_P, DT * _P], f16, kind="ExternalInput")

    t_pos = nc.dram_tensor("pos_out", [_P, DT], f32, kind="ExternalOutput")
    t_neg = nc.dram_tensor("neg_out", [_P, DT], f32, kind="ExternalOutput")

    t_ar_in = nc.dram_tensor("ar_in", [H], f32)
    t_ar_out = nc.dram_tensor("ar_out", [H], f32, addr_space="Shared")

    tab_lo = t_tab[0:_LO, :]
    tab_hi = t_tab[_LO:N, :]

    with tile.TileContext(nc) as tc:
        import contextlib

        ctx = contextlib.ExitStack()
        consts = ctx.enter_context(tc.tile_pool(name="consts", bufs=1))
        glo = ctx.enter_context(tc.tile_pool(name="glo", bufs=GBUFS))
        ghi = ctx.enter_context(tc.tile_pool(name="ghi", bufs=GBUFS))
        stp = ctx.enter_context(tc.tile_pool(name="stp", bufs=4))
        aggps = ctx.enter_context(tc.tile_pool(name="aggps", bufs=2, space="PSUM"))
        trps = ctx.enter_context(tc.tile_pool(name="trps", bufs=2, space="PSUM"))
        zps = ctx.enter_context(tc.tile_pool(name="zps", bufs=2, space="PSUM"))
        ep = ctx.enter_context(tc.tile_pool(name="ep", bufs=3))
        misc = ctx.enter_context(tc.tile_pool(name="misc", bufs=2))
        miscps = ctx.enter_context(tc.tile_pool(name="miscps", bufs=1, space="PSUM"))
        csps = ctx.enter_context(tc.tile_pool(name="csps", bufs=1, space="PSUM"))

        # ---- constants ----
        W0 = consts.tile([_P, H], f16, tag="W0")
        W1 = consts.tile([_P, H], f16, tag="W1")
        nc.sync.dma_start(W0[:], t_W[0:_P, :])
        nc.sync.dma_start(W1[:], t_W[_P : 2 * _P, :])
        stwself_sb = consts.tile([_P, DT * _P], f16, tag="stwself")
        nc.sync.dma_start(stwself_sb[:], t_stwself[:])
        iota4_t = consts.tile([_P, 4, _P], f16, tag="iota4")
        nc.sync.dma_start(iota4_t[:], t_iota4[:].rearrange("p (t q) -> p t q", t=4))
        ident16 = consts.tile([_P, _P], f16, tag="ident16")
        nc.sync.dma_start(ident16[:], t_ident16[:])
        mask16 = consts.tile([_P, DT], f16, tag="mask16")
        nc.sync.dma_start(mask16[:], t_mask[:])
        b_sb = consts.tile([1, H], f32, tag="b_sb")
        nc.sync.dma_start(b_sb[:], t_b[None, :])
        dwT0 = consts.tile([_P, H], f32, tag="dwT0")
        dwT1 = consts.tile([_P, H], f32, tag="dwT1")
        nc.sync.dma_start(dwT0[:], t_dwT[0:_P, :])
        nc.sync.dma_start(dwT1[:], t_dwT[_P : 2 * _P, :])
        ones_row = consts.tile([1, _P], f32, tag="ones_row")
        nc.vector.memset(ones_row[:], 1.0)
        ones_col = consts.tile([_P, 1], f32, tag="ones_col")
        nc.vector.memset(ones_col[:], 1.0)

        # ---- stream loads ----
        i1_sb = consts.tile([_P, n_et * 8], mybir.dt.int16, tag="i1")
        w1_sb = consts.tile([_P, n_et], f16, tag="w1")
        d1_sb = consts.tile([_P, n_et], f16, tag="d1")
        nc.sync.dma_start(i1_sb[:], t_i1[:])
        nc.sync.dma_start(w1_sb[:], t_w1[:])
        nc.sync.dma_start(d1_sb[:], t_d1[:])

        # ---- persistent z tiles + summary accumulator ----
        zbuf1 = consts.tile([_P, DT * H], f16, tag="zbuf1")
        zbuf2 = consts.tile([_P, DT * H], f16, tag="zbuf2")
        cs_acc = csps.tile([1, H], f32, tag="cs_acc")

        qctr = [0]

        # ---- main sweep: one pass over dst tiles serves both encodings --
        for dti in range(DT):
            Tl, Th = int(T1[dti, 0]), int(T1[dti, 1])
            gl = gh = None
            CH = int(os.environ.get("KV_CH", "4"))  # gather chunk (tiles)
            gs = ep.tile([_P, F2], f16, tag="gs")
            nc.sync.dma_start(gs[:], t_tabself[dti * _P : (dti + 1) * _P, :])
            if Tl:
                o = int(O1[dti, 0])
                gl = glo.tile([_P, max_Tl, F2], f16, tag="gl")
                for c0 in range(0, Tl, CH):
                    c1 = min(Tl, c0 + CH)
                    nc.gpsimd.dma_gather(
                        gl[:, c0:c1, :],
                        tab_lo,
                        i1_sb[:, 8 * (o + c0) : 8 * (o + c1)],
                        (c1 - c0) * _P,
                        (c1 - c0) * _P,
                        F2,
                        single_packet=((c1 - c0) * _P <= 1024),
                        queue_num=qctr[0] % NQ,
                    )
                    qctr[0] += 1
            if Th:
                o = int(O1[dti, 1])
                gh = ghi.tile([_P, max_Th, F2], f16, tag="gh")
                for c0 in range(0, Th, CH):
                    c1 = min(Th, c0 + CH)
                    nc.gpsimd.dma_gather(
                        gh[:, c0:c1, :],
                        tab_hi,
                        i1_sb[:, 8 * (o + c0) : 8 * (o + c1)],
                        (c1 - c0) * _P,
                        (c1 - c0) * _P,
                        F2,
                        single_packet=((c1 - c0) * _P <= 1024),
                        queue_num=qctr[0] % NQ,
                    )
                    qctr[0] += 1

            ps = aggps.tile([_P, F2], f32, tag="aggps")
            n_mm = Tl + Th + 1
            # self-loop contribution: precomputed diag(dinv^2) one-hot
            nc.tensor.matmul(
                ps[:], stwself_sb[:, dti * _P : (dti + 1) * _P], gs[:],
                start=True, stop=False,
            )
            k = 1
            for Tn, g, o0 in ((Tl, gl, int(O1[dti, 0])), (Th, gh, int(O1[dti, 1]))):
                j = 0
                while j < Tn:
                    pw = min(4, Tn - j)
                    t = o0 + j
                    eq = stp.tile([_P, 4, _P], f16, tag="eq")
                    stw = stp.tile([_P, 4, _P], f16, tag="stw")
                    nc.vector.tensor_tensor(
                        eq[:, :pw, :],
                        d1_sb[:, t : t + pw].to_broadcast([_P, pw, _P]),
                        iota4_t[:, :pw, :],
                        mybir.AluOpType.is_equal,
                    )
                    nc.vector.tensor_tensor(
                        stw[:, :pw, :],
                        eq[:, :pw, :],
                        w1_sb[:, t : t + pw].to_broadcast([_P, pw, _P]),
                        mybir.AluOpType.mult,
                    )
                    for q in range(pw):
                        nc.tensor.matmul(
                            ps[:],
                            stw[:, q, :],
                            g[:, j + q, :],
                            start=False,
                            stop=(k == n_mm - 1),
                        )
                        k += 1
                    j += pw

            # epilogue: agg [128, 512] = [agg1 | agg2] -> z1, z2 tiles
            agg_sb = ep.tile([_P, F2], f16, tag="agg_sb")
            nc.any.tensor_copy(agg_sb[:], ps[:])
            zp = zps.tile([_P, F2], f32, tag="zp")
            for half in range(2):
                nc.tensor.matmul(
                    zp[:, half * H : (half + 1) * H], ones_row[:], b_sb[:],
                    start=True, stop=False,
                )
            for k4 in range(4):
                tp = trps.tile([_P, _P], f16, tag="trps")
                nc.tensor.transpose(
                    tp[:], agg_sb[:, k4 * _P : (k4 + 1) * _P], ident16[:]
                )
                aggT = ep.tile([_P, _P], f16, tag="aggT")
                nc.any.tensor_copy(aggT[:], tp[:])
                half = 0 if k4 < 2 else 1
                Wk = W0 if (k4 % 2) == 0 else W1
                nc.tensor.matmul(
                    zp[:, half * H : (half + 1) * H], aggT[:], Wk[:],
                    start=False, stop=((k4 % 2) == 1),
                )

            # PReLU straight to f16 stores (scalar engine)
            nc.scalar.activation(
                zbuf1[:, dti * H : (dti + 1) * H], zp[:, 0:H],
                mybir.ActivationFunctionType.Prelu, alpha=a_val,
            )
            nc.scalar.activation(
                zbuf2[:, dti * H : (dti + 1) * H], zp[:, H:F2],
                mybir.ActivationFunctionType.Prelu, alpha=a_val,
            )
            # column-sum accumulate on PE (occupancy mask kills empty slots)
            nc.tensor.matmul(
                cs_acc[:], mask16[:, dti : dti + 1],
                zbuf1[:, dti * H : (dti + 1) * H],
                start=(dti == 0), stop=(dti == DT - 1),
            )

        # ---- summary + AllReduce ----
        cs_sb = misc.tile([1, H], f32, tag="cs_sb")
        nc.vector.tensor_copy(cs_sb[:], cs_acc[:])
        nc.sync.dma_start(t_ar_in[None, :], cs_sb[:])
        nc.gpsimd.collective_compute(
            "AllReduce",
            mybir.AluOpType.add,
            replica_groups=[list(range(C))],
            ins=[t_ar_in[:]],
            outs=[t_ar_out[:]],
        )
        sums_sb = misc.tile([1, H], f32, tag="sums_sb")
        nc.sync.dma_start(sums_sb[:], t_ar_out[None, :])
        summ_sb = misc.tile([1, H], f32, tag="summ_sb")
        nc.scalar.activation(
            summ_sb[:], sums_sb[:], mybir.ActivationFunctionType.Sigmoid,
            scale=1.0 / N,
        )

        # ---- wsum = disc_W @ summary ----
        ident = consts.tile([_P, _P], f32, tag="ident")
        nc.sync.dma_start(ident[:], t_ident[:])
        sT = misc.tile([_P, 2], f32, tag="sT")
        for c_i in range(2):
            tp = miscps.tile([_P, _P], f32, tag="mps")
            nc.tensor.transpose(
                tp[:, 0:1],
                summ_sb[0:1, c_i * _P : (c_i + 1) * _P],
                ident[0:1, 0:1],
            )
            nc.vector.tensor_copy(sT[:, c_i : c_i + 1], tp[:, 0:1])
        ws_ps = miscps.tile([1, H], f32, tag="mps")
        nc.tensor.matmul(ws_ps[:], sT[:, 0:1], dwT0[:], start=True, stop=False)
        nc.tensor.matmul(ws_ps[:], sT[:, 1:2], dwT1[:], start=False, stop=True)
        ws_sb = misc.tile([1, H], f32, tag="ws_sb")
        nc.vector.tensor_copy(ws_sb[:], ws_ps[:])
        wb_ps = miscps.tile([_P, H], f32, tag="mps")
        nc.tensor.matmul(wb_ps[:], ones_row[:], ws_sb[:], start=True, stop=True)
        wsum_bc = consts.tile([_P, H], f16, tag="wsum_bc")
        nc.vector.tensor_copy(wsum_bc[:], wb_ps[:])

        # ---- pos/neg dots ----
        pos_acc = consts.tile([_P, DT], f32, tag="pos_acc")
        neg_acc = consts.tile([_P, DT], f32, tag="neg_acc")
        DB = 7  # dot batch (DT=49 = 7*7)
        scratch = misc.tile([_P, DB, H], f16, tag="scratch")
        for zbuf, acc in ((zbuf1, pos_acc), (zbuf2, neg_acc)):
            for d0 in range(0, DT, DB):
                k = min(DB, DT - d0)
                zv = zbuf[:, d0 * H : (d0 + k) * H].rearrange(
                    "p (t h) -> p t h", t=k
                )
                nc.vector.tensor_tensor(
                    scratch[:, :k, :], zv,
                    wsum_bc[:].rearrange("p (o h) -> p o h", o=1).to_broadcast([_P, k, H]),
                    mybir.AluOpType.mult,
                )
                nc.vector.reduce_sum(
                    acc[:, d0 : d0 + k], scratch[:, :k, :], bass_rust.AxisListType.X
                )

        nc.sync.dma_start(t_pos[:], pos_acc[:])
        nc.sync.dma_start(t_neg[:], neg_acc[:])
        ctx.close()

    nc.compile()

    in_maps = []
    for c in range(C):
        in_maps.append(
            {
                "tab": tab,
                "w16": W_f16,
                "bvec": b,
                "dwT": dwT,
                "mask16": occ[c],
                "ident_in": np.eye(_P, dtype=np.float32),
                "ident16": ident16_np,
                "idx1": i1[c],
                "wgt1": w1[c],
                "dstl1": d1[c],
                "iota4": iota4_np,
                "tabself": tab_self[c],
                "stwself": stwself[c],
            }
        )

    if os.environ.get("KERNEL_SIM", "0") == "1":
        from concourse import bass_interp

        sim = bass_interp.MultiCoreSim(nc, C)
        for c in range(C):
            for k, v in in_maps[c].items():
                sim.cores[c].tensor(k)[:] = v
        sim.simulate()
        results = [
            {
                "pos_out": np.array(sim.cores[c].tensor("pos_out")),
                "neg_out": np.array(sim.cores[c].tensor("neg_out")),
            }
            for c in range(C)
        ]
    else:
        trace = os.environ.get("KERNEL_TRACE", "0") == "1"
        kw = {}
        if trace:
            kw["trace"] = True
        res = run_bass_kernel_spmd(nc, in_maps, core_ids=list(range(C)), **kw)
        kernel.last_result = res
        results = res.results

    pos_s = np.concatenate(
        [results[c]["pos_out"].T.reshape(-1) for c in range(C)]
    )
    neg_s = np.concatenate(
        [results[c]["neg_out"].T.reshape(-1) for c in range(C)]
    )
    return pos_s[slot_of_node].astype(np.float32), neg_s[slot_of_node].astype(
        np.float32
    )


# revision 19
# speedup vs baseline: 1.2126x; 1.0023x over previous
"""DGI (Deep Graph Infomax) Trainium2 kernel — v2.

Strategy (8 NeuronCores, one shared SPMD program):
  - Nodes sharded by destination: core c owns dst nodes [c*N/8, (c+1)*N/8).
  - Aggregate-then-multiply: z = PReLU((A_hat @ x) @ W + b).  The x-space
    aggregation needs no precomputed xw, so gathers start immediately.
  - Paired table tab[i] = [x[i] | x[perm[i]]] (f16, 1KB rows).  dma_gather
    cost is row-count-bound (~8ns/row, flat 256B..1KB), so one gather per
    edge fetches BOTH passes' source rows.  One shared weighted one-hot
    (fused is_equal+mult on DVE) and one PE matmul [128e,128d]^T@[128e,512]
    accumulate [agg1|agg2] per dst tile in PSUM.
  - Gathers round-robin over 4 SWDGE queues (~2x DMA overlap).
  - Epilogue per dst tile: PSUM->SBUF f16, 2+2 PE transposes, (aggT @ W)
    k-accumulated, bias+PReLU on DVE; z1/z2 tiles stored f16 in SBUF;
    z1 column-sums accumulate for the summary.
  - summary = sigmoid(mean over all z1) via ones-matmul column reduce +
    1KB AllReduce; wsum = disc_W @ summary on PE; broadcast via K=1 matmul.
  - pos/neg = z . wsum per tile on DVE; per-core [128, DT] outputs,
    host unshards.
"""

import os

import numpy as np

_P = 128
_LO = 32768
_C = 8


def _build_streams(sidx, ed, ew, C, NS, DT):
    """Per-core gather/weight/dstl streams with shared tile structure.

    Returns (idx_sbuf [C,128,n_et*8] i16, w_sbuf [C,128,n_et] f32,
             dl_sbuf [C,128,n_et] f32, Tmax [DT,2], off_tiles [DT,2], n_et)
    """
    core = ed // NS
    ldst = ed - core * NS
    dt = ldst // _P
    dstl = ldst % _P
    cls = (sidx >= _LO).astype(np.int64)

    gid = (core * DT + dt) * 2 + cls
    NG = C * DT * 2
    cnt = np.bincount(gid, minlength=NG).reshape(C, DT, 2)
    T = -(-cnt // _P)
    Tmax = T.max(axis=0)
    flat = Tmax.reshape(-1)
    off_tiles = np.concatenate([[0], np.cumsum(flat)[:-1]]).reshape(DT, 2)
    n_et = int(flat.sum())

    order = np.argsort(gid, kind="stable")
    sorted_gid = gid[order]
    g_starts = np.concatenate(
        [[0], np.cumsum(np.bincount(sorted_gid, minlength=NG))[:-1]]
    )
    rank = np.arange(order.size) - g_starts[sorted_gid]
    g_dt = (sorted_gid // 2) % DT
    g_cls = sorted_gid % 2
    pos = off_tiles[g_dt, g_cls] * _P + rank
    core_s = sorted_gid // (DT * 2)

    L = n_et * _P
    idx16 = np.zeros((C, L), np.int16)
    wv = np.zeros((C, L), np.float16)
    dl = np.full((C, L), -1.0, np.float16)  # int16-able dstl, -1 = pad
    sidx_s = sidx[order]
    idx16[core_s, pos] = (sidx_s - g_cls * _LO).astype(np.int16)
    wv[core_s, pos] = ew[order].astype(np.float16)
    dl[core_s, pos] = dstl[order].astype(np.float16)

    idx_w = idx16.reshape(C, L // 16, 16).transpose(0, 2, 1)
    idx_sbuf = np.ascontiguousarray(np.tile(idx_w, (1, 8, 1)))
    w_sbuf = np.ascontiguousarray(wv.reshape(C, n_et, _P).transpose(0, 2, 1))
    dl_sbuf = np.ascontiguousarray(dl.reshape(C, n_et, _P).transpose(0, 2, 1))
    return idx_sbuf, w_sbuf, dl_sbuf, Tmax, off_tiles, n_et


def kernel(x, W, b, a, disc_W, edge_index, perm):
    import bass_rust
    import concourse.bacc as bacc
    import concourse.mybir as mybir
    import concourse.tile as tile
    from concourse.bass_utils import run_bass_kernel_spmd

    x = np.asarray(x)
    W = np.asarray(W)
    b = np.asarray(b, np.float32)
    a = np.asarray(a, np.float32)
    disc_W = np.asarray(disc_W, np.float32)
    ei = np.asarray(edge_index, np.int64)
    perm_np = np.asarray(perm, np.int64)

    N, F = x.shape
    H = W.shape[1]
    C = _C
    NS = N // C
    DT = -(-NS // _P)
    LAST = NS - (DT - 1) * _P
    F2 = 2 * F  # paired row width
    f16 = mybir.dt.float16
    f32 = mybir.dt.float32
    NQ = 4  # SWDGE queues
    GBUFS = 2

    # ---- host preprocessing -------------------------------------------
    src = ei[0]
    dst = ei[1]
    deg = (np.bincount(dst, minlength=N) + 1.0).astype(np.float32)
    dinv = (1.0 / np.sqrt(deg)).astype(np.float32)
    ew = dinv[src] * dinv[dst]

    # balanced node->slot remap: deal nodes (sorted by in-edge count, desc)
    # serpentine over C*DT bins of 128 slots -> near-equal edges per tile
    NBINS = C * DT
    NSL = DT * _P  # slots per core
    cnt_in = np.bincount(dst, minlength=N)
    order = np.argsort(-cnt_in, kind="stable")
    ii = np.arange(N, dtype=np.int64)
    rnd = ii // NBINS
    pos_in_rnd = ii % NBINS
    bins = np.where(rnd % 2 == 0, pos_in_rnd, NBINS - 1 - pos_in_rnd)
    slot_sorted = (bins // DT) * NSL + (bins % DT) * _P + rnd
    slot_of_node = np.empty(N, np.int64)
    slot_of_node[order] = slot_sorted
    node_at_slot = np.full(C * NSL, -1, np.int64)
    node_at_slot[slot_of_node] = np.arange(N, dtype=np.int64)

    dst_s = slot_of_node[dst]
    i1, w1, d1, T1, O1, n_et = _build_streams(src, dst_s, ew, C, NSL, DT)
    a_val = float(np.asarray(a).reshape(-1)[0])
    occ = np.zeros((C, _P, DT), np.float16)
    stwself = np.zeros((C, DT, _P, _P), np.float16)
    dsq = dinv * dinv
    nas = node_at_slot.reshape(C, DT, _P)
    for c in range(C):
        for dti in range(DT):
            nn = nas[c, dti]
            valid = nn >= 0
            idxs = np.nonzero(valid)[0]
            stwself[c, dti, idxs, idxs] = dsq[nn[idxs]].astype(np.float16)
            occ[c, valid, dti] = 1.0
    stwself = np.ascontiguousarray(
        stwself.transpose(0, 2, 1, 3).reshape(C, _P, DT * _P)
    )

    x_f16 = x.astype(np.float16)
    tab = np.ascontiguousarray(np.concatenate([x_f16, x_f16[perm_np]], axis=1))
    tab_self = np.zeros((C, DT * _P, 2 * F), np.float16)
    flat = node_at_slot.reshape(C, DT * _P)
    for c in range(C):
        valid = flat[c] >= 0
        tab_self[c, valid] = tab[flat[c][valid]]
    W_f16 = np.ascontiguousarray(W.astype(np.float16))
    dwT = np.ascontiguousarray(disc_W.T.astype(np.float32))
    iota4_np = np.tile(np.arange(_P, dtype=np.float16)[None, :], (_P, 4))
    ident16_np = np.eye(_P, dtype=np.float16)
    iota4_np = np.tile(np.arange(_P, dtype=np.float16)[None, :], (_P, 4))

    max_Tl = max(int(T1[:, 0].max()), 1)
    max_Th = max(int(T1[:, 1].max()), 1)

    # ---- device program -----------------------------------------------
    nc = bacc.Bacc(
        "TRN2", target_bir_lowering=False, debug=False, num_devices=C,
        num_swdge_queues=NQ,
        dynamic_dma_scratch_size=32768,
    )

    t_tab = nc.dram_tensor("tab", [N, F2], f16, kind="ExternalInput")
    t_W = nc.dram_tensor("w16", [F, H], f16, kind="ExternalInput")
    t_b = nc.dram_tensor("bvec", [H], f32, kind="ExternalInput")
    t_dwT = nc.dram_tensor("dwT", [H, H], f32, kind="ExternalInput")

    t_mask = nc.dram_tensor("mask16", [_P, DT], f16, kind="ExternalInput")
    t_ident = nc.dram_tensor("ident_in", [_P, _P], f32, kind="ExternalInput")
    t_ident16 = nc.dram_tensor("ident16", [_P, _P], f16, kind="ExternalInput")
    t_i1 = nc.dram_tensor("idx1", [_P, n_et * 8], mybir.dt.int16, kind="ExternalInput")
    t_w1 = nc.dram_tensor("wgt1", [_P, n_et], f16, kind="ExternalInput")
    t_d1 = nc.dram_tensor("dstl1", [_P, n_et], f16, kind="ExternalInput")
    t_iota4 = nc.dram_tensor("iota4", [_P, 4 * _P], f16, kind="ExternalInput")
    t_tabself = nc.dram_tensor("tabself", [DT * _P, F2], f16, kind="ExternalInput")
    t_stwself = nc.dram_tensor("stwself", [_P, DT * _P], f16, kind="ExternalInput")

    t_pos = nc.dram_tensor("pos_out", [_P, DT], f32, kind="ExternalOutput")
    t_neg = nc.dram_tensor("neg_out", [_P, DT], f32, kind="ExternalOutput")

    t_ar_in = nc.dram_tensor("ar_in", [H], f32)
    t_ar_out = nc.dram_tensor("ar_out", [H], f32, addr_space="Shared")

    tab_lo = t_tab[0:_LO, :]
    tab_hi = t_tab[_LO:N, :]

    with tile.TileContext(nc) as tc:
        import contextlib

        ctx = contextlib.ExitStack()
        consts = ctx.enter_context(tc.tile_pool(name="consts", bufs=1))
        glo = ctx.enter_context(tc.tile_pool(name="glo", bufs=GBUFS))
        ghi = ctx.enter_context(tc.tile_pool(name="ghi", bufs=GBUFS))
        stp = ctx.enter_context(tc.tile_pool(name="stp", bufs=4))
        aggps = ctx.enter_context(tc.tile_pool(name="aggps", bufs=2, space="PSUM"))
        trps = ctx.enter_context(tc.tile_pool(name="trps", bufs=2, space="PSUM"))
        zps = ctx.enter_context(tc.tile_pool(name="zps", bufs=2, space="PSUM"))
        ep = ctx.enter_context(tc.tile_pool(name="ep", bufs=3))
        misc = ctx.enter_context(tc.tile_pool(name="misc", bufs=2))
        miscps = ctx.enter_context(tc.tile_pool(name="miscps", bufs=1, space="PSUM"))
        csps = ctx.enter_context(tc.tile_pool(name="csps", bufs=1, space="PSUM"))

        # ---- constants ----
        W0 = consts.tile([_P, H], f16, tag="W0")
        W1 = consts.tile([_P, H], f16, tag="W1")
        nc.sync.dma_start(W0[:], t_W[0:_P, :])
        nc.sync.dma_start(W1[:], t_W[_P : 2 * _P, :])
        stwself_sb = consts.tile([_P, DT * _P], f16, tag="stwself")
        nc.sync.dma_start(stwself_sb[:], t_stwself[:])
        iota4_t = consts.tile([_P, 4, _P], f16, tag="iota4")
        nc.sync.dma_start(iota4_t[:], t_iota4[:].rearrange("p (t q) -> p t q", t=4))
        ident16 = consts.tile([_P, _P], f16, tag="ident16")
        nc.sync.dma_start(ident16[:], t_ident16[:])
        mask16 = consts.tile([_P, DT], f16, tag="mask16")
        nc.sync.dma_start(mask16[:], t_mask[:])
        b_sb = consts.tile([1, H], f32, tag="b_sb")
        nc.sync.dma_start(b_sb[:], t_b[None, :])
        dwT0 = consts.tile([_P, H], f32, tag="dwT0")
        dwT1 = consts.tile([_P, H], f32, tag="dwT1")
        nc.sync.dma_start(dwT0[:], t_dwT[0:_P, :])
        nc.sync.dma_start(dwT1[:], t_dwT[_P : 2 * _P, :])
        ones_row = consts.tile([1, _P], f32, tag="ones_row")
        nc.vector.memset(ones_row[:], 1.0)
        ones_col = consts.tile([_P, 1], f32, tag="ones_col")
        nc.vector.memset(ones_col[:], 1.0)

        # ---- stream loads ----
        i1_sb = consts.tile([_P, n_et * 8], mybir.dt.int16, tag="i1")
        w1_sb = consts.tile([_P, n_et], f16, tag="w1")
        d1_sb = consts.tile([_P, n_et], f16, tag="d1")
        nc.sync.dma_start(i1_sb[:], t_i1[:])
        nc.sync.dma_start(w1_sb[:], t_w1[:])
        nc.sync.dma_start(d1_sb[:], t_d1[:])

        # ---- persistent z tiles + summary accumulator ----
        zbuf1 = consts.tile([_P, DT * H], f16, tag="zbuf1")
        zbuf2 = consts.tile([_P, DT * H], f16, tag="zbuf2")
        cs_acc = csps.tile([1, H], f32, tag="cs_acc")

        qctr = [0]

        # ---- main sweep: one pass over dst tiles serves both encodings --
        for dti in range(DT):
            Tl, Th = int(T1[dti, 0]), int(T1[dti, 1])
            gl = gh = None
            CH = 4  # gather chunk (tiles per dma_gather instruction)
            gs = ep.tile([_P, F2], f16, tag="gs")
            nc.sync.dma_start(gs[:], t_tabself[dti * _P : (dti + 1) * _P, :])
            if Tl:
                o = int(O1[dti, 0])
                gl = glo.tile([_P, max_Tl, F2], f16, tag="gl")
                for c0 in range(0, Tl, CH):
                    c1 = min(Tl, c0 + CH)
                    nc.gpsimd.dma_gather(
                        gl[:, c0:c1, :],
                        tab_lo,
                        i1_sb[:, 8 * (o + c0) : 8 * (o + c1)],
                        (c1 - c0) * _P,
                        (c1 - c0) * _P,
                        F2,
                        single_packet=((c1 - c0) * _P <= 1024),
                        queue_num=qctr[0] % NQ,
                    )
                    qctr[0] += 1
            if Th:
                o = int(O1[dti, 1])
                gh = ghi.tile([_P, max_Th, F2], f16, tag="gh")
                for c0 in range(0, Th, CH):
                    c1 = min(Th, c0 + CH)
                    nc.gpsimd.dma_gather(
                        gh[:, c0:c1, :],
                        tab_hi,
                        i1_sb[:, 8 * (o + c0) : 8 * (o + c1)],
                        (c1 - c0) * _P,
                        (c1 - c0) * _P,
                        F2,
                        single_packet=((c1 - c0) * _P <= 1024),
                        queue_num=qctr[0] % NQ,
                    )
                    qctr[0] += 1

            ps = aggps.tile([_P, F2], f32, tag="aggps")
            n_mm = Tl + Th + 1
            # self-loop contribution: precomputed diag(dinv^2) one-hot
            nc.tensor.matmul(
                ps[:], stwself_sb[:, dti * _P : (dti + 1) * _P], gs[:],
                start=True, stop=False,
            )
            k = 1
            for Tn, g, o0 in ((Tl, gl, int(O1[dti, 0])), (Th, gh, int(O1[dti, 1]))):
                j = 0
                while j < Tn:
                    pw = min(4, Tn - j)
                    t = o0 + j
                    eq = stp.tile([_P, 4, _P], f16, tag="eq")
                    stw = stp.tile([_P, 4, _P], f16, tag="stw")
                    nc.vector.tensor_tensor(
                        eq[:, :pw, :],
                        d1_sb[:, t : t + pw].to_broadcast([_P, pw, _P]),
                        iota4_t[:, :pw, :],
                        mybir.AluOpType.is_equal,
                    )
                    nc.vector.tensor_tensor(
                        stw[:, :pw, :],
                        eq[:, :pw, :],
                        w1_sb[:, t : t + pw].to_broadcast([_P, pw, _P]),
                        mybir.AluOpType.mult,
                    )
                    for q in range(pw):
                        nc.tensor.matmul(
                            ps[:],
                            stw[:, q, :],
                            g[:, j + q, :],
                            start=False,
                            stop=(k == n_mm - 1),
                        )
                        k += 1
                    j += pw

            # epilogue: agg [128, 512] = [agg1 | agg2] -> z1, z2 tiles
            agg_sb = ep.tile([_P, F2], f16, tag="agg_sb")
            nc.any.tensor_copy(agg_sb[:], ps[:])
            zp = zps.tile([_P, F2], f32, tag="zp")
            for half in range(2):
                nc.tensor.matmul(
                    zp[:, half * H : (half + 1) * H], ones_row[:], b_sb[:],
                    start=True, stop=False,
                )
            for k4 in range(4):
                tp = trps.tile([_P, _P], f16, tag="trps")
                nc.tensor.transpose(
                    tp[:], agg_sb[:, k4 * _P : (k4 + 1) * _P], ident16[:]
                )
                aggT = ep.tile([_P, _P], f16, tag="aggT")
                nc.any.tensor_copy(aggT[:], tp[:])
                half = 0 if k4 < 2 else 1
                Wk = W0 if (k4 % 2) == 0 else W1
                nc.tensor.matmul(
                    zp[:, half * H : (half + 1) * H], aggT[:], Wk[:],
                    start=False, stop=((k4 % 2) == 1),
                )

            # PReLU straight to f16 stores (scalar engine)
            nc.scalar.activation(
                zbuf1[:, dti * H : (dti + 1) * H], zp[:, 0:H],
                mybir.ActivationFunctionType.Prelu, alpha=a_val,
            )
            nc.scalar.activation(
                zbuf2[:, dti * H : (dti + 1) * H], zp[:, H:F2],
                mybir.ActivationFunctionType.Prelu, alpha=a_val,
            )
            # column-sum accumulate on PE (occupancy mask kills empty slots)
            nc.tensor.matmul(
                cs_acc[:], mask16[:, dti : dti + 1],
                zbuf1[:, dti * H : (dti + 1) * H],
                start=(dti == 0), stop=(dti == DT - 1),
            )

        # ---- summary + AllReduce ----
        cs_sb = misc.tile([1, H], f32, tag="cs_sb")
        nc.vector.tensor_copy(cs_sb[:], cs_acc[:])
        nc.sync.dma_start(t_ar_in[None, :], cs_sb[:])
        nc.gpsimd.collective_compute(
            "AllReduce",
            mybir.AluOpType.add,
            replica_groups=[list(range(C))],
            ins=[t_ar_in[:]],
            outs=[t_ar_out[:]],
        )
        sums_sb = misc.tile([1, H], f32, tag="sums_sb")
        nc.sync.dma_start(sums_sb[:], t_ar_out[None, :])
        summ_sb = misc.tile([1, H], f32, tag="summ_sb")
        nc.scalar.activation(
            summ_sb[:], sums_sb[:], mybir.ActivationFunctionType.Sigmoid,
            scale=1.0 / N,
        )

        # ---- wsum = disc_W @ summary ----
        ident = consts.tile([_P, _P], f32, tag="ident")
        nc.sync.dma_start(ident[:], t_ident[:])
        sT = misc.tile([_P, 2], f32, tag="sT")
        for c_i in range(2):
            tp = miscps.tile([_P, _P], f32, tag="mps")
            nc.tensor.transpose(
                tp[:, 0:1],
                summ_sb[0:1, c_i * _P : (c_i + 1) * _P],
                ident[0:1, 0:1],
            )
            nc.vector.tensor_copy(sT[:, c_i : c_i + 1], tp[:, 0:1])
        ws_ps = miscps.tile([1, H], f32, tag="mps")
        nc.tensor.matmul(ws_ps[:], sT[:, 0:1], dwT0[:], start=True, stop=False)
        nc.tensor.matmul(ws_ps[:], sT[:, 1:2], dwT1[:], start=False, stop=True)
        ws_sb = misc.tile([1, H], f32, tag="ws_sb")
        nc.vector.tensor_copy(ws_sb[:], ws_ps[:])
        wb_ps = miscps.tile([_P, H], f32, tag="mps")
        nc.tensor.matmul(wb_ps[:], ones_row[:], ws_sb[:], start=True, stop=True)
        wsum_bc = consts.tile([_P, H], f16, tag="wsum_bc")
        nc.vector.tensor_copy(wsum_bc[:], wb_ps[:])

        # ---- pos/neg dots ----
        pos_acc = consts.tile([_P, DT], f32, tag="pos_acc")
        neg_acc = consts.tile([_P, DT], f32, tag="neg_acc")
        DB = 7  # dot batch (DT=49 = 7*7)
        scratch = misc.tile([_P, DB, H], f16, tag="scratch")
        for zbuf, acc in ((zbuf1, pos_acc), (zbuf2, neg_acc)):
            for d0 in range(0, DT, DB):
                k = min(DB, DT - d0)
                zv = zbuf[:, d0 * H : (d0 + k) * H].rearrange(
                    "p (t h) -> p t h", t=k
                )
                nc.vector.tensor_tensor(
                    scratch[:, :k, :], zv,
                    wsum_bc[:].rearrange("p (o h) -> p o h", o=1).to_broadcast([_P, k, H]),
                    mybir.AluOpType.mult,
                )
                nc.vector.reduce_sum(
                    acc[:, d0 : d0 + k], scratch[:, :k, :], bass_rust.AxisListType.X
                )

        nc.sync.dma_start(t_pos[:], pos_acc[:])
        nc.sync.dma_start(t_neg[:], neg_acc[:])
        ctx.close()

    nc.compile()

    in_maps = []
    for c in range(C):
        in_maps.append(
            {
                "tab": tab,
                "w16": W_f16,
                "bvec": b,
                "dwT": dwT,
                "mask16": occ[c],
                "ident_in": np.eye(_P, dtype=np.float32),
                "ident16": ident16_np,
                "idx1": i1[c],
                "wgt1": w1[c],
                "dstl1": d1[c],
                "iota4": iota4_np,
                "tabself": tab_self[c],
                "stwself": stwself[c],
            }
        )

    if os.environ.get("KERNEL_SIM", "0") == "1":
        from concourse import bass_interp

        sim = bass_interp.MultiCoreSim(nc, C)
        for c in range(C):
            for k, v in in_maps[c].items():
                sim.cores[c].tensor(k)[:] = v
        sim.simulate()
        results = [
            {
                "pos_out": np.array(sim.cores[c].tensor("pos_out")),
                "neg_out": np.array(sim.cores[c].tensor("neg_out")),
            }
            for c in range(C)
        ]
    else:
        trace = os.environ.get("KERNEL_TRACE", "0") == "1"
        kw = {}
        if trace:
            kw["trace"] = True
        res = run_bass_kernel_spmd(nc, in_maps, core_ids=list(range(C)), **kw)
        kernel.last_result = res
        results = res.results

    pos_s = np.concatenate(
        [results[c]["pos_out"].T.reshape(-1) for c in range(C)]
    )
    neg_s = np.concatenate(
        [results[c]["neg_out"].T.reshape(-1) for c in range(C)]
    )
    return pos_s[slot_of_node].astype(np.float32), neg_s[slot_of_node].astype(
        np.float32
    )
